# revision 19
# baseline (speedup 1.0000x reference)
"""Trainium2 Bass kernel for gathered-row MLP decode matmul.

out[b, 0, r] = sum_d x[b, 0, d] * weight[indices[r], d]

Strategy: dedup+sort the indices on the host and shard them contiguously
across 8 cores (~452 rows each, padded to a 128-multiple NV with
duplicate row-0 indices — transpose-mode dma_gather requires
num_idxs%128==0). The fp32 weight is converted to fp16 on the host (the
2e-2 relative-error budget gives ~80x headroom over fp16 quantization
noise, measured 2.4e-4). Each core gathers its NV rows with
dma_gather(transpose=True) in 128-column chunks, landing them directly
in matmul-ready [d%128, d//128, r] layout. The matmul streams the BATCH
dim (N=32) with the gathered rows as the stationary operand:
out[r, b] += whiT[:, k, r].T @ xT[:, k*B:(k+1)*B], so PE time is
32 rows/ktile instead of NV — the kernel is purely
gather-bandwidth-bound (cost model: ~360B/ns aggregate DMA). The last
chunk is gathered as two half-row (2048-elem) gathers through a
rearranged [2*D_FF, 2048] view so its first 16 k-tiles of matmuls
overlap the final transfer, halving the post-DMA tail chain. Each
chunk's [128, 32] PSUM result is copied to SBUF by the (otherwise idle)
DVE and DMA'd out per chunk; the host transposes and scatters back to
the original 4403-index order.
"""

import os
import sys
from contextlib import ExitStack

sys.path.insert(0, "/opt/trn_rl_repo")
os.environ.setdefault("MYCRO_LOCAL_CACHE", "1")

import numpy as np

D_FF = 11008
D_MODEL = 4096
R_TOTAL = 4403
B = 32
NCORES = 8
P = 128
KT = D_MODEL // P  # 32 contraction tiles

_cache = {}


def _build(nv):
    """Build the SPMD program for a padded per-core index count of nv."""
    key = ("tmm", nv)
    if key in _cache:
        return _cache[key]
    from concourse import bacc, mybir, tile

    f32 = mybir.dt.float32
    f16 = mybir.dt.float16
    i16 = mybir.dt.int16

    assert nv % P == 0
    nch = nv // P
    # gather idx layout (wrapped-16 columns): nch-1 full chunks, then the
    # last chunk as low/high half-row index blocks
    idx_cols = (nch + 1) * 8

    nc = bacc.Bacc(
        "TRN2",
        target_bir_lowering=False,
        debug=False,
        enable_asserts=False,
    )
    w_dram = nc.dram_tensor("whi", [D_FF, D_MODEL], f16, kind="ExternalInput").ap()
    xh_dram = nc.dram_tensor("xh", [P, KT * B], f16, kind="ExternalInput").ap()
    idx_dram = nc.dram_tensor("idx", [P, idx_cols], i16, kind="ExternalInput").ap()
    out_dram = nc.dram_tensor("out", [P, nch * B], f32, kind="ExternalOutput").ap()
    # half-row view for the split last chunk: [2*D_FF, 2048]
    w_half = w_dram.rearrange("a (b c) -> (a b) c", b=2)

    with tile.TileContext(nc) as tc, ExitStack() as ctx:
        consts = ctx.enter_context(tc.tile_pool(name="consts", bufs=1))
        whi_pool = ctx.enter_context(tc.tile_pool(name="whiT", bufs=min(nch, 8)))
        half_pool = ctx.enter_context(tc.tile_pool(name="whiH", bufs=2))
        psum = ctx.enter_context(tc.tile_pool(name="psum", bufs=4, space="PSUM"))
        out_pool = ctx.enter_context(tc.tile_pool(name="outp", bufs=1))
        out_sb = out_pool.tile([P, nch * B], f32)

        # idx first: the gathers (the critical path) depend only on it
        idx_sb = consts.tile([P, idx_cols], i16)
        nc.sync.dma_start(idx_sb[:], idx_dram)
        xh_sb = consts.tile([P, KT * B], f16)
        nc.sync.dma_start(xh_sb[:], xh_dram)

        def chain(ps, whiT, k0, k1, koff):
            for k in range(k0, k1):
                nc.tensor.matmul(
                    out=ps[:],
                    lhsT=whiT[:, k - koff, :],
                    rhs=xh_sb[:, k * B : (k + 1) * B],
                    start=(k == 0),
                    stop=(k == KT - 1),
                )

        def store(c, ps):
            nc.vector.tensor_copy(out_sb[:, c * B : (c + 1) * B], ps[:])

        for c in range(nch - 1):
            # whiT[p, k, i] = whi[idx[c*128+i], k*128 + p]
            whiT = whi_pool.tile([P, KT, P], f16, tag="whiT")
            nc.gpsimd.dma_gather(
                out_ap=whiT[:],
                in_ap=w_dram,
                idxs_ap=idx_sb[:, c * 8 : (c + 1) * 8],
                num_idxs=P,
                num_idxs_reg=P,
                elem_size=D_MODEL,
                transpose=True,
            )
            # out[r, b] = sum_k whiT[:, k, :].T @ x[:, k] — batch (32) is the
            # streamed dim, the gathered rows are the stationary operand.
            ps = psum.tile([P, B], mybir.dt.float32, tag="ps")
            chain(ps, whiT, 0, KT, 0)
            store(c, ps)

        # Last chunk split into low/high half-row gathers so its first 16
        # k-tiles of matmuls overlap the second half's transfer, halving the
        # post-DMA tail chain.
        ps = psum.tile([P, B], mybir.dt.float32, tag="ps")
        for h in range(2):
            whiH = half_pool.tile([P, KT // 2, P], f16, tag="whiH")
            nc.gpsimd.dma_gather(
                out_ap=whiH[:],
                in_ap=w_half,
                idxs_ap=idx_sb[:, (nch - 1 + h) * 8 : (nch + h) * 8],
                num_idxs=P,
                num_idxs_reg=P,
                elem_size=D_MODEL // 2,
                transpose=True,
            )
            chain(ps, whiH, h * (KT // 2), (h + 1) * (KT // 2), h * (KT // 2))
        store(nch - 1, ps)

        for c in range(nch):
            nc.sync.dma_start(
                out_dram[:, c * B : (c + 1) * B], out_sb[:, c * B : (c + 1) * B]
            )

    nc.compile()
    _cache[key] = nc
    return nc


def _make_in_maps(x, weight, indices):
    """Returns (in_maps, assemble_fn, nv)."""
    x = np.asarray(x, dtype=np.float32)
    weight = np.asarray(weight, dtype=np.float32)
    indices = np.asarray(indices, dtype=np.int64)

    whi = np.ascontiguousarray(weight.astype(np.float16))

    # x^T staged so the DMA is contiguous: xh[p, k*32+b] = x[b, 0, k*128+p]
    xh = np.ascontiguousarray(
        x[:, 0, :]
        .reshape(B, KT, P)
        .transpose(2, 1, 0)
        .reshape(P, KT * B)
        .astype(np.float16)
    )

    uniq, inv = np.unique(indices, return_inverse=True)
    nu = len(uniq)
    base, rem = divmod(nu, NCORES)
    counts = [base + (1 if c < rem else 0) for c in range(NCORES)]
    starts = np.concatenate([[0], np.cumsum(counts)[:-1]])
    nv = max(-(-max(counts) // P) * P, 2 * P)
    nch = nv // P

    def _wrap16(v):
        """[n] int16 -> wrapped-16 [16, n//16] block."""
        return v.reshape(-1, 16).T

    in_maps = []
    for c in range(NCORES):
        idx_pad = np.zeros(nv, dtype=np.int16)
        idx_pad[: counts[c]] = uniq[starts[c] : starts[c] + counts[c]]
        last = idx_pad[(nch - 1) * P :].astype(np.int16)
        cols = [
            _wrap16(idx_pad[: (nch - 1) * P]),  # full chunks
            _wrap16(2 * last),  # last chunk, low halves
            _wrap16(2 * last + 1),  # last chunk, high halves
        ]
        blk = np.concatenate(cols, axis=1)  # [16, idx_cols]
        in_maps.append(
            {
                "whi": whi,
                "xh": xh,
                "idx": np.ascontiguousarray(np.tile(blk, (8, 1))),
            }
        )

    def assemble(results):
        vals = np.empty((nu, B), dtype=np.float32)
        for c in range(NCORES):
            o = results[c]["out"]  # [128, nch*B]
            core_rows = np.concatenate(
                [o[:, i * B : (i + 1) * B] for i in range(nch)], axis=0
            )  # [nv, B]
            vals[starts[c] : starts[c] + counts[c]] = core_rows[: counts[c]]
        out = vals[inv].T  # [B, len(indices)]
        return np.ascontiguousarray(out.reshape(B, 1, len(inv)))

    return in_maps, assemble, nv


def _filter_in_maps(nc, in_maps):
    names = set()
    from concourse import mybir

    for alloc in nc.m.functions[0].allocations:
        if isinstance(alloc, mybir.MemoryLocationSet) and alloc.kind == "ExternalInput":
            names.add(alloc.memorylocations[0].name)
    return [{k: v for k, v in m.items() if k in names} for m in in_maps]


def run_full(x, weight, indices, trace=False):
    """Returns (output, BassKernelResults)."""
    from concourse.bass_utils import run_bass_kernel_spmd

    in_maps, assemble, nv = _make_in_maps(x, weight, indices)
    nc = _build(nv)
    in_maps = _filter_in_maps(nc, in_maps)
    res = run_bass_kernel_spmd(nc, in_maps, list(range(NCORES)), trace=trace)
    return assemble(res.results), res


def kernel(x, weight, indices):
    out, _ = run_full(x, weight, indices)
    return out
